# revision 1
# baseline (speedup 1.0000x reference)
# Trainium2 Bass kernel for nn_Member_Aggregator (GNN attention aggregation).
#
# Math (per edge e with node n = segment(e), 32 edges/node):
#   e_u   = u2e[neigh_idx]                          [E, 64]
#   g_rep = g2e[nodes][seg]                         [E, 64]
#   h1    = relu(e_u @ W1a.T + g_rep @ W1b.T + b1)  [E, 64]   (att1_w = [W1a | W1b])
#   h2    = relu(h1 @ W2.T + b2)                    [E, 64]
#   lg    = h2 @ w3.T (+ b3, dropped: softmax-invariant)
#   att   = segment_softmax(lg); out[n] = sum att * e_u        [N, 64]
#
# Sharding: 5000 contiguous nodes per core (x8), tables+weights replicated.
#
# Per-core layout ("stacked" feature-major): nodes padded to 5120 = 40 blocks
# x 128 nodes. Block = 4 tiles x 1024 edges. A tile pairs nodes {16t..16t+15}
# (top, SBUF partitions 0..63) with {64+16t..} (bottom, partitions 64..127),
# so every [128, 512] activation column holds one top edge + one bottom edge
# and all matmuls use block-diagonal weights at full 128-partition width.
# Edge slot x in [0,1024): x = c*128 + p (gather chunk c, partition p);
# top x = 32*j + k (node-slot j, neighbor k), bottom x-512 likewise.
#
# Per-edge q = g_rep @ W1b.T + b1 is folded into mm1 as extra contraction rows
# (lhsT = transposed per-node q, rhs = constant node-indicator), so no
# per-edge vector add is needed.

import os
import sys

import numpy as np

for _p in ("/opt/trn_rl_repo",):
    if _p not in sys.path:
        sys.path.insert(0, _p)

N_NODES = 40000
DEG = 32
D = 64
NUM_USERS = 100000
NUM_GROUPS = 50000
N_CORES = 8
NPC = N_NODES // N_CORES  # 5000 nodes per core
TPB = 4                   # tiles per block
EPT = 1024                # edges per tile

_cache = {}


def _build_program(nblk):
    """Build the SPMD per-core Bass program for `nblk` 128-node blocks."""
    import concourse.bass as bass
    import concourse.tile as tile
    from concourse import bacc, mybir
    from concourse.bass import IndirectOffsetOnAxis
    from contextlib import ExitStack

    f32 = mybir.dt.float32
    bf16 = mybir.dt.bfloat16
    i32 = mybir.dt.int32
    AF = mybir.ActivationFunctionType
    ALU = mybir.AluOpType
    AX = mybir.AxisListType

    npad = nblk * 128
    ntile = nblk * TPB

    nc = bacc.Bacc("TRN2", target_bir_lowering=False, debug=False,
                   num_devices=N_CORES)

    u2e = nc.dram_tensor("u2e", [NUM_USERS, D], bf16, kind="ExternalInput").ap()
    g2e = nc.dram_tensor("g2e", [NUM_GROUPS, D], f32, kind="ExternalInput").ap()
    eidx = nc.dram_tensor("eidx", [ntile * 128, 8], i32, kind="ExternalInput").ap()
    gidx = nc.dram_tensor("gidx", [nblk * 64, 2], i32, kind="ExternalInput").ap()
    w1a_d = nc.dram_tensor("w1a", [128, 128], bf16, kind="ExternalInput").ap()
    w1b_d = nc.dram_tensor("w1b", [128, 128], f32, kind="ExternalInput").ap()
    w2_d = nc.dram_tensor("w2", [128, 128], bf16, kind="ExternalInput").ap()
    w3_d = nc.dram_tensor("w3q", [128, TPB * 8], bf16, kind="ExternalInput").ap()
    ones_d = nc.dram_tensor("onesbd", [2, 128], bf16, kind="ExternalInput").ap()
    ind_d = nc.dram_tensor("ind64", [64, TPB * 512], bf16,
                           kind="ExternalInput").ap()
    b1_d = nc.dram_tensor("b1st", [128, 1], f32, kind="ExternalInput").ap()
    b2_d = nc.dram_tensor("b2st", [128, 1], f32, kind="ExternalInput").ap()
    id_d = nc.dram_tensor("ident", [128, 128], f32, kind="ExternalInput").ap()
    idb_d = nc.dram_tensor("identb", [128, 128], bf16, kind="ExternalInput").ap()
    outd = nc.dram_tensor("out", [npad, D], f32, kind="ExternalOutput").ap()

    with tile.TileContext(nc) as tc, ExitStack() as ctx:
        cp = ctx.enter_context(tc.tile_pool(name="consts", bufs=1))

        def load_const(dram_ap, shape, tag, dt=f32):
            t = cp.tile(shape, dt, tag=tag)
            nc.gpsimd.dma_start(t[:], dram_ap)
            return t

        w1a_t = load_const(w1a_d, [128, 128], "w1a", bf16)
        w1b_t = load_const(w1b_d, [128, 128], "w1b")
        w2_t = load_const(w2_d, [128, 128], "w2", bf16)
        w3_t = load_const(w3_d, [128, TPB * 8], "w3", bf16)
        ones_t = load_const(ones_d, [2, 128], "ones", bf16)
        ind_t = load_const(ind_d, [64, TPB * 512], "ind", bf16)
        b1_t = load_const(b1_d, [128, 1], "b1")
        b2_t = load_const(b2_d, [128, 1], "b2")
        id_t = load_const(id_d, [128, 128], "ident")
        idb_t = load_const(idb_d, [128, 128], "identb", bf16)

        gq = ctx.enter_context(tc.tile_pool(name="gq", bufs=2))
        qps = ctx.enter_context(tc.tile_pool(name="qpsum", bufs=2, space="PSUM"))
        eip = ctx.enter_context(tc.tile_pool(name="ei", bufs=3))
        gep = ctx.enter_context(tc.tile_pool(name="ge", bufs=3))
        tpps = ctx.enter_context(tc.tile_pool(name="tp", bufs=2, space="PSUM"))
        eut = ctx.enter_context(tc.tile_pool(name="eut", bufs=6))
        mmps = ctx.enter_context(tc.tile_pool(name="mm", bufs=2, space="PSUM"))
        hsb = ctx.enter_context(tc.tile_pool(name="h", bufs=3))
        lgps = ctx.enter_context(tc.tile_pool(name="lg", bufs=1, space="PSUM"))
        abps = ctx.enter_context(tc.tile_pool(name="attb", bufs=1, space="PSUM"))
        lrow_p = ctx.enter_context(tc.tile_pool(name="lrow", bufs=2))
        arow_p = ctx.enter_context(tc.tile_pool(name="arow", bufs=2))
        nm = ctx.enter_context(tc.tile_pool(name="nm", bufs=2))
        wsb_p = ctx.enter_context(tc.tile_pool(name="w", bufs=2))
        wacc_p = ctx.enter_context(tc.tile_pool(name="wacc", bufs=2))
        osb_p = ctx.enter_context(tc.tile_pool(name="osb", bufs=2))

        for b in range(nblk):
            # logits psum for the whole block: partition 2t+h = (tile t, half h)
            lg8 = lgps.tile([8, 512], f32)
            # ---- q phase: per-node q = g2e[gid] @ W1b.T + b1, transposed ----
            # gidx row p = (node p, node 64+p): gathered [64, 2, 64] chunk has
            # per-partition [g(n) | g(n+64)]; its transpose is the stacked g2T.
            gi = gq.tile([64, 2], i32, tag="gi")
            nc.gpsimd.dma_start(gi[:], gidx[b * 64:(b + 1) * 64, :])
            gt = gq.tile([64, 128], f32, tag="gt")
            for c in range(2):
                nc.gpsimd.indirect_dma_start(
                    out=gt[:, D * c:D * (c + 1)],
                    out_offset=None, in_=g2e,
                    in_offset=IndirectOffsetOnAxis(ap=gi[:, c:c + 1], axis=0))
            g2T = qps.tile([128, 128], f32, tag="qp")
            nc.tensor.transpose(out=g2T[:, 0:64], in_=gt[:],
                                identity=id_t[0:64, 0:64])
            g2T_sb = gq.tile([128, D], f32, tag="g2Tsb")
            nc.scalar.copy(g2T_sb[:], g2T[:, 0:64])
            qp = qps.tile([128, 128], f32, tag="qp")
            nc.tensor.matmul(qp[:, 0:64], lhsT=w1b_t[:], rhs=g2T_sb[:],
                             start=True, stop=True)
            q2T_sb = gq.tile([128, D], f32, tag="q2T")
            nc.vector.tensor_scalar_add(q2T_sb[:], qp[:, 0:64], b1_t[:, :1])
            qT2p = qps.tile([128, 128], f32, tag="qp")
            nc.tensor.transpose(out=qT2p[0:64, :], in_=q2T_sb[:], identity=id_t[:])
            qT2_sb = gq.tile([64, 128], bf16, tag="qT2")
            nc.scalar.copy(qT2_sb[:], qT2p[0:64, :])

            # ---- edge phase ----
            euts = []
            for t in range(TPB):
                ti = b * TPB + t
                ei = eip.tile([128, 8], i32)
                nc.gpsimd.dma_start(ei[:], eidx[ti * 128:(ti + 1) * 128, :])
                ge = gep.tile([128, 512], bf16)
                for c in range(8):
                    nc.gpsimd.indirect_dma_start(
                        out=ge[:, D * c:D * (c + 1)],
                        out_offset=None, in_=u2e,
                        in_offset=IndirectOffsetOnAxis(ap=ei[:, c:c + 1], axis=0))
                # chunks interleave (top, bottom) rows per partition, so each
                # [128, 128] transpose writes the stacked layout at base 0
                tp = tpps.tile([128, 512], bf16)
                for u in range(4):
                    nc.tensor.transpose(
                        out=tp[:, 128 * u:128 * (u + 1)],
                        in_=ge[:, 128 * u:128 * (u + 1)], identity=idb_t[:])
                eut_sb = eut.tile([128, 512], bf16)
                nc.scalar.copy(eut_sb[:], tp[:])
                euts.append(eut_sb)

                h1p = mmps.tile([128, 512], f32, tag="mm")
                nc.tensor.matmul(h1p[:], lhsT=(w1a_t[:]),
                                 rhs=(eut_sb[:]), start=True, stop=False)
                nc.tensor.matmul(h1p[:], lhsT=(qT2_sb[:]),
                                 rhs=(ind_t[:, t * 512:(t + 1) * 512]),
                                 start=False, stop=True)
                h1sb = hsb.tile([128, 512], bf16, tag="h")
                nc.scalar.activation(h1sb[:], h1p[:], AF.Relu)
                h2p = mmps.tile([128, 512], f32, tag="mm")
                nc.tensor.matmul(h2p[:], lhsT=(w2_t[:]),
                                 rhs=(h1sb[:]), start=True, stop=True)
                h2sb = hsb.tile([128, 512], bf16, tag="h")
                nc.scalar.activation(h2sb[:], h2p[:], AF.Relu, bias=b2_t[:, :1])
                nc.tensor.matmul(lg8[:], lhsT=(w3_t[:, 8 * t:8 * (t + 1)]),
                                 rhs=(h2sb[:]), start=(t == 0),
                                 stop=(t == TPB - 1))

            # ---- softmax over each node's 32 edges (node-major [128, 32]) ----
            lrow = lrow_p.tile([8, 512], f32)
            nc.scalar.copy(lrow[:], lg8[:])
            lnm = nm.tile([128, 32], f32, tag="lnm")
            for t in range(TPB):
                for h in range(2):
                    nc.gpsimd.dma_start(
                        lnm[64 * h + 16 * t:64 * h + 16 * t + 16, :],
                        lrow[2 * t + h:2 * t + h + 1, :]
                            .rearrange("p (j k) -> p j k", j=16))
            ngmax = nm.tile([128, 1], f32, tag="ngmax")
            nc.vector.tensor_reduce(out=ngmax[:], in_=lnm[:], axis=AX.X,
                                    op=ALU.max, negate=True)
            expn = nm.tile([128, 32], f32, tag="expn")
            sume = nm.tile([128, 1], f32, tag="sume")
            nc.scalar.activation(expn[:], lnm[:], AF.Exp, bias=ngmax[:, :1],
                                 accum_out=sume[:, :1])
            rinv = nm.tile([128, 1], f32, tag="rinv")
            nc.vector.reciprocal(rinv[:], sume[:])
            attn = nm.tile([128, 32], bf16, tag="attn")
            nc.vector.tensor_scalar_mul(attn[:], expn[:], rinv[:, :1])
            arow = arow_p.tile([2, TPB * 512], bf16)
            for t in range(TPB):
                for h in range(2):
                    nc.gpsimd.dma_start(
                        arow[h:h + 1, 512 * t:512 * (t + 1)]
                            .rearrange("p (j k) -> p j k", j=16),
                        attn[64 * h + 16 * t:64 * h + 16 * t + 16, :])

            # ---- weighted aggregation ----
            wacc = wacc_p.tile([128, D], f32)
            for t in range(TPB):
                ab = abps.tile([128, 512], f32)
                nc.tensor.matmul(ab[:], lhsT=(ones_t[:]),
                                 rhs=(arow[:, t * 512:(t + 1) * 512]),
                                 start=True, stop=True)
                wt = wsb_p.tile([128, 512], f32)
                nc.vector.tensor_tensor(out=wt[:], in0=euts[t][:], in1=ab[:],
                                        op=ALU.mult)
                nc.vector.tensor_reduce(
                    out=wacc[:, 16 * t:16 * (t + 1)],
                    in_=wt[:].rearrange("p (j k) -> p j k", j=16),
                    axis=AX.X, op=ALU.add)
            outp = qps.tile([128, 128], f32, tag="qp")
            nc.tensor.transpose(out=outp[0:64, :], in_=wacc[:], identity=id_t[:])
            osb = osb_p.tile([64, 128], f32)
            nc.scalar.copy(osb[:], outp[0:64, :])
            nc.gpsimd.dma_start(
                outd[b * 128:(b + 1) * 128, :]
                    .rearrange("(pair n) d -> n pair d", pair=2),
                osb[:].rearrange("n (pair d) -> n pair d", pair=2))

    nc.compile()
    return nc


def _prep_host(nodes, neigh_idx, att1_w, att1_b, att2_w, att2_b, att3_w,
               nblk_per_core):
    """Shard + reorder indices, build constant tensors. Returns per-core maps
    (without the shared tables)."""
    npad = nblk_per_core * 128
    npc = min(NPC, npad)
    nodes = np.asarray(nodes).astype(np.int32)
    neigh = np.asarray(neigh_idx).astype(np.int32).reshape(-1, DEG)

    consts = {}
    att1_w = np.asarray(att1_w, np.float32)
    w1aT = att1_w[:, :D].T.copy()
    w1bT = att1_w[:, D:].T.copy()
    w2T = np.asarray(att2_w, np.float32).T.copy()

    def blockdiag(m):
        z = np.zeros((128, 128), np.float32)
        z[:64, :64] = m
        z[64:, 64:] = m
        return z

    import ml_dtypes
    bf = ml_dtypes.bfloat16
    consts["w1a"] = blockdiag(w1aT).astype(bf)
    consts["w1b"] = blockdiag(w1bT)
    consts["w2"] = blockdiag(w2T).astype(bf)
    # w3q[:, t*8 + 2t + h] = w3 half-h; tile t's mm3 writes lg8 rows 2t, 2t+1
    w3q = np.zeros((128, TPB, 8), np.float32)
    w3row = np.asarray(att3_w, np.float32)[0]
    for t in range(TPB):
        w3q[:64, t, 2 * t] = w3row
        w3q[64:, t, 2 * t + 1] = w3row
    consts["w3q"] = w3q.reshape(128, TPB * 8).astype(bf)
    ones_bd = np.zeros((2, 128), np.float32)
    ones_bd[0, :64] = 1.0
    ones_bd[1, 64:] = 1.0
    consts["onesbd"] = ones_bd.astype(bf)
    # ind64[j, t*512 + e] = 1 iff j == 16t + e//32 (mm1b scatters per-node q)
    ind64 = np.zeros((64, TPB * 512), np.float32)
    for t in range(TPB):
        ind64[16 * t:16 * (t + 1), 512 * t:512 * (t + 1)] = np.repeat(
            np.eye(16, dtype=np.float32), 32, axis=1)
    consts["ind64"] = ind64.astype(bf)
    consts["b1st"] = np.tile(np.asarray(att1_b, np.float32), 2)[:, None].copy()
    consts["b2st"] = np.tile(np.asarray(att2_b, np.float32), 2)[:, None].copy()
    consts["ident"] = np.eye(128, dtype=np.float32)
    consts["identb"] = np.eye(128, dtype=np.float32).astype(bf)

    ncores = len(nodes) // npc if len(nodes) >= npc else 1
    per_core = []
    for c in range(ncores):
        n0 = c * npc
        nix = np.zeros((npad, DEG), np.int32)
        nix[:npc] = neigh[n0:n0 + npc]
        gid = np.zeros(npad, np.int32)
        gid[:npc] = nodes[n0:n0 + npc]
        # [b, n_local(128), k] -> [b, h, t, j, k] -> [b, t, h, j, k]
        a = nix.reshape(nblk_per_core, 2, TPB, 16, DEG).transpose(0, 2, 1, 3, 4)
        # flat x = 512h + 128u + p; gather chunk order interleaves (top,
        # bottom): eidx[.., p, 2u + h] = edge (h, u, p)
        a = a.reshape(nblk_per_core, TPB, 2, 4, 128).transpose(0, 1, 4, 3, 2)
        eidx = np.ascontiguousarray(a.reshape(nblk_per_core * TPB * 128, 8))
        # gidx row (b, p) = (node p, node 64+p) of block b
        gi2 = np.ascontiguousarray(
            gid.reshape(nblk_per_core, 2, 64).transpose(0, 2, 1)
               .reshape(nblk_per_core * 64, 2))
        m = dict(consts)
        m["eidx"] = eidx
        m["gidx"] = gi2
        per_core.append(m)
    return per_core


def kernel(nodes, neigh_idx, segment_ids, u2e_weight, g2e_weight,
           att1_w, att1_b, att2_w, att2_b, att3_w, att3_b):
    from concourse import bass_utils

    nblk = NPC // 128 + (1 if NPC % 128 else 0)  # 40
    key = ("prog", nblk)
    if key not in _cache:
        _cache[key] = _build_program(nblk)
    nc = _cache[key]

    import ml_dtypes
    u2e = np.ascontiguousarray(
        np.asarray(u2e_weight, np.float32).astype(ml_dtypes.bfloat16))
    g2e = np.ascontiguousarray(np.asarray(g2e_weight, np.float32))
    per_core = _prep_host(nodes, neigh_idx, att1_w, att1_b, att2_w, att2_b,
                          att3_w, nblk)
    in_maps = []
    for m in per_core:
        m = dict(m)
        m["u2e"] = u2e
        m["g2e"] = g2e
        in_maps.append(m)

    res = bass_utils.run_bass_kernel_spmd(nc, in_maps,
                                          core_ids=list(range(N_CORES)))
    outs = [np.asarray(r["out"])[:NPC] for r in res.results]
    return np.concatenate(outs, axis=0)



# revision 25
# speedup vs baseline: 1.0737x; 1.0737x over previous
# Trainium2 Bass kernel for nn_Member_Aggregator (GNN attention aggregation).
#
# Math (per edge e with node n = segment(e), 32 edges/node):
#   e_u   = u2e[neigh_idx]                          [E, 64]
#   g_rep = g2e[nodes][seg]                         [E, 64]
#   h1    = relu(e_u @ W1a.T + g_rep @ W1b.T + b1)  [E, 64]   (att1_w = [W1a | W1b])
#   h2    = relu(h1 @ W2.T + b2)                    [E, 64]
#   lg    = h2 @ w3.T (+ b3, dropped: softmax-invariant)
#   att   = segment_softmax(lg); out[n] = sum att * e_u        [N, 64]
#
# Sharding: 5000 contiguous nodes per core (x8), tables+weights replicated.
#
# Per-core layout ("stacked" feature-major): nodes padded to 5120 = 40 blocks
# x 128 nodes. Block = 4 tiles x 1024 edges. A tile pairs nodes {16t..16t+15}
# (top, SBUF partitions 0..63) with {64+16t..} (bottom, partitions 64..127),
# so every [128, 512] activation column holds one top edge + one bottom edge
# and all matmuls use block-diagonal weights at full 128-partition width.
# Edge slot x in [0,1024): x = c*128 + p (gather chunk c, partition p);
# top x = 32*j + k (node-slot j, neighbor k), bottom x-512 likewise.
#
# Edge embeddings are fetched with ONE dma_gather per block (4096 int16
# indices, vectorized Q7 descriptor generation) from a host-compacted
# per-block table (the block's <=4096 unique u2e rows, f32 so each row is a
# 256B gather element). This sidesteps both the 1-index-per-partition limit
# of indirect_dma_start and dma_gather's int16 index range.
#
# Per-edge q = g_rep @ W1b.T + b1 is folded into mm1 as extra contraction rows
# (lhsT = transposed per-node q, rhs = constant node-indicator), so no
# per-edge vector add is needed.

import os
import sys

import numpy as np

for _p in ("/opt/trn_rl_repo",):
    if _p not in sys.path:
        sys.path.insert(0, _p)

N_NODES = 40000
DEG = 32
D = 64
NUM_USERS = 100000
NUM_GROUPS = 50000
N_CORES = 8
NPC = N_NODES // N_CORES  # 5000 nodes per core
TPB = 4                   # tiles per block
EPT = 1024                # edges per tile
EPB = TPB * EPT           # 4096 edges per block (= compact table rows)

_cache = {}


def _build_program(nblk):
    """Build the SPMD per-core Bass program for `nblk` 128-node blocks."""
    import concourse.bass as bass
    import concourse.tile as tile
    from concourse import bacc, mybir
    from concourse.bass import IndirectOffsetOnAxis
    from contextlib import ExitStack

    f32 = mybir.dt.float32
    bf16 = mybir.dt.bfloat16
    i32 = mybir.dt.int32
    i16 = mybir.dt.int16
    AF = mybir.ActivationFunctionType
    ALU = mybir.AluOpType
    AX = mybir.AxisListType

    npad = nblk * 128
    ntile = nblk * TPB

    nc = bacc.Bacc("TRN2", target_bir_lowering=False, debug=False,
                   num_devices=N_CORES)

    ctab = nc.dram_tensor("ctab", [nblk * EPB, D], f32,
                          kind="ExternalInput").ap()
    g2e = nc.dram_tensor("g2e", [NUM_GROUPS, D], f32, kind="ExternalInput").ap()
    eidx = nc.dram_tensor("eidx", [nblk * 128, EPB // 16], i16,
                          kind="ExternalInput").ap()
    gidx = nc.dram_tensor("gidx", [nblk * 64, 2], i32, kind="ExternalInput").ap()
    w1a_d = nc.dram_tensor("w1a", [128, 128], bf16, kind="ExternalInput").ap()
    w1b_d = nc.dram_tensor("w1b", [128, 128], f32, kind="ExternalInput").ap()
    w2_d = nc.dram_tensor("w2", [128, 128], bf16, kind="ExternalInput").ap()
    w3_d = nc.dram_tensor("w3q", [128, TPB * 8], bf16, kind="ExternalInput").ap()
    ones_d = nc.dram_tensor("onesbd", [2, 128], bf16, kind="ExternalInput").ap()
    ind_d = nc.dram_tensor("ind64", [64, TPB * 512], bf16,
                           kind="ExternalInput").ap()
    b1_d = nc.dram_tensor("b1st", [128, 1], f32, kind="ExternalInput").ap()
    b2_d = nc.dram_tensor("b2st", [128, 1], f32, kind="ExternalInput").ap()
    id_d = nc.dram_tensor("ident", [128, 128], f32, kind="ExternalInput").ap()
    outd = nc.dram_tensor("out", [npad, D], f32, kind="ExternalOutput").ap()

    with tile.TileContext(nc) as tc, ExitStack() as ctx:
        cp = ctx.enter_context(tc.tile_pool(name="consts", bufs=1))

        def load_const(dram_ap, shape, tag, dt=f32):
            t = cp.tile(shape, dt, tag=tag)
            nc.sync.dma_start(t[:], dram_ap)
            return t

        w1a_t = load_const(w1a_d, [128, 128], "w1a", bf16)
        w1b_t = load_const(w1b_d, [128, 128], "w1b")
        w2_t = load_const(w2_d, [128, 128], "w2", bf16)
        w3_t = load_const(w3_d, [128, TPB * 8], "w3", bf16)
        ones_t = load_const(ones_d, [2, 128], "ones", bf16)
        ind_t = load_const(ind_d, [64, TPB * 512], "ind", bf16)
        b1_t = load_const(b1_d, [128, 1], "b1")
        b2_t = load_const(b2_d, [128, 1], "b2")
        id_t = load_const(id_d, [128, 128], "ident")

        gq = ctx.enter_context(tc.tile_pool(name="gq", bufs=2))
        qps = ctx.enter_context(tc.tile_pool(name="qpsum", bufs=2, space="PSUM"))
        eip = ctx.enter_context(tc.tile_pool(name="ei", bufs=3))
        gep = ctx.enter_context(tc.tile_pool(name="ge", bufs=3))
        tpps = ctx.enter_context(tc.tile_pool(name="tp", bufs=2, space="PSUM"))
        eut = ctx.enter_context(tc.tile_pool(name="eut", bufs=6))
        mmps = ctx.enter_context(tc.tile_pool(name="mm", bufs=2, space="PSUM"))
        hsb = ctx.enter_context(tc.tile_pool(name="h", bufs=3))
        lgps = ctx.enter_context(tc.tile_pool(name="lg", bufs=1, space="PSUM"))
        abps = ctx.enter_context(tc.tile_pool(name="attb", bufs=1, space="PSUM"))
        lrow_p = ctx.enter_context(tc.tile_pool(name="lrow", bufs=2))
        arow_p = ctx.enter_context(tc.tile_pool(name="arow", bufs=2))
        nm = ctx.enter_context(tc.tile_pool(name="nm", bufs=2))
        wsb_p = ctx.enter_context(tc.tile_pool(name="w", bufs=2))
        wacc_p = ctx.enter_context(tc.tile_pool(name="wacc", bufs=2))
        osb_p = ctx.enter_context(tc.tile_pool(name="osb", bufs=2))

        for b in range(nblk):
            # logits psum for the whole block: partition 2t+h = (tile t, half h)
            lg8 = lgps.tile([8, 512], f32)
            # ---- q phase: per-node q = g2e[gid] @ W1b.T + b1, transposed ----
            # gidx row p = (node p, node 64+p): gathered [64, 2, 64] chunk has
            # per-partition [g(n) | g(n+64)]; its transpose is the stacked g2T.
            gi = gq.tile([64, 2], i32, tag="gi")
            nc.sync.dma_start(gi[:], gidx[b * 64:(b + 1) * 64, :])
            gt = gq.tile([64, 128], f32, tag="gt")
            nc.gpsimd.indirect_dma_start(
                out=gt[:], out_offset=None, in_=g2e,
                in_offset=IndirectOffsetOnAxis(ap=gi[:], axis=0))
            g2T = qps.tile([128, 128], f32, tag="qp")
            nc.tensor.transpose(out=g2T[:, 0:64], in_=gt[:],
                                identity=id_t[0:64, 0:64])
            g2T_sb = gq.tile([128, D], f32, tag="g2Tsb")
            nc.scalar.copy(g2T_sb[:], g2T[:, 0:64])
            qp = qps.tile([128, 128], f32, tag="qp")
            nc.tensor.matmul(qp[:, 0:64], lhsT=w1b_t[:], rhs=g2T_sb[:],
                             start=True, stop=True)
            q2T_sb = gq.tile([128, D], f32, tag="q2T")
            nc.vector.tensor_scalar_add(q2T_sb[:], qp[:, 0:64], b1_t[:, :1])
            qT2p = qps.tile([128, 128], f32, tag="qp")
            nc.tensor.transpose(out=qT2p[0:64, :], in_=q2T_sb[:], identity=id_t[:])
            qT2_sb = gq.tile([64, 128], bf16, tag="qT2")
            nc.scalar.copy(qT2_sb[:], qT2p[0:64, :])

            # ---- edge phase: one dma_gather for the whole block ----
            ei = eip.tile([128, EPB // 16], i16)
            nc.sync.dma_start(ei[:], eidx[b * 128:(b + 1) * 128, :])
            ge = gep.tile([128, TPB * 512], f32)
            # single_packet caps at 64 descs/lane = 1024 idx; multi-packet
            # mode handles the full 4096-idx block gather
            nc.gpsimd.dma_gather(
                out_ap=ge[:].rearrange("p (c f) -> p c f", f=D),
                in_ap=ctab[b * EPB:(b + 1) * EPB, :],
                idxs_ap=ei[:], num_idxs=EPB, num_idxs_reg=EPB, elem_size=D,
                single_packet=False)
            euts = []
            for t in range(TPB):
                # chunks interleave (top, bottom) rows per partition, so each
                # [128, 128] transpose writes the stacked layout at base 0
                tp = tpps.tile([128, 512], f32)
                for u in range(4):
                    nc.tensor.transpose(
                        out=tp[:, 128 * u:128 * (u + 1)],
                        in_=ge[:, t * 512 + 128 * u:t * 512 + 128 * (u + 1)],
                        identity=id_t[:])
                eut_sb = eut.tile([128, 512], bf16)
                nc.scalar.copy(eut_sb[:], tp[:])
                euts.append(eut_sb)

                h1p = mmps.tile([128, 512], f32, tag="mm")
                nc.tensor.matmul(h1p[:], lhsT=(w1a_t[:]),
                                 rhs=(eut_sb[:]), start=True, stop=False)
                nc.tensor.matmul(h1p[:], lhsT=(qT2_sb[:]),
                                 rhs=(ind_t[:, t * 512:(t + 1) * 512]),
                                 start=False, stop=True)
                h1sb = hsb.tile([128, 512], bf16, tag="h")
                nc.scalar.activation(h1sb[:], h1p[:], AF.Relu)
                h2p = mmps.tile([128, 512], f32, tag="mm")
                nc.tensor.matmul(h2p[:], lhsT=(w2_t[:]),
                                 rhs=(h1sb[:]), start=True, stop=True)
                h2sb = hsb.tile([128, 512], bf16, tag="h")
                nc.scalar.activation(h2sb[:], h2p[:], AF.Relu, bias=b2_t[:, :1])
                nc.tensor.matmul(lg8[:], lhsT=(w3_t[:, 8 * t:8 * (t + 1)]),
                                 rhs=(h2sb[:]), start=(t == 0),
                                 stop=(t == TPB - 1))

            # ---- softmax over each node's 32 edges (node-major [128, 32]) ----
            lrow = lrow_p.tile([8, 512], f32)
            nc.scalar.copy(lrow[:], lg8[:])
            lnm = nm.tile([128, 32], f32, tag="lnm")
            # lg8 row q = 4h + t (set via w3q), so ravel orders line up:
            # lnm[64h+16t+j, k] = lrow[4h+t, 32j+k] in one partition-fan DMA
            nc.sync.dma_start(
                lnm[:],
                lrow[:].rearrange("q (j k) -> q j k", j=16))
            ngmax = nm.tile([128, 1], f32, tag="ngmax")
            nc.vector.tensor_reduce(out=ngmax[:], in_=lnm[:], axis=AX.X,
                                    op=ALU.max, negate=True)
            expn = nm.tile([128, 32], f32, tag="expn")
            sume = nm.tile([128, 1], f32, tag="sume")
            nc.scalar.activation(expn[:], lnm[:], AF.Exp, bias=ngmax[:, :1],
                                 accum_out=sume[:, :1])
            rinv = nm.tile([128, 1], f32, tag="rinv")
            nc.vector.reciprocal(rinv[:], sume[:])
            attn = nm.tile([128, 32], bf16, tag="attn")
            nc.vector.tensor_scalar_mul(attn[:], expn[:], rinv[:, :1])
            arow = arow_p.tile([2, TPB * 512], bf16)
            # arow[h, 512t+32j+k] = attn[64h+16t+j, k] in one partition-fan DMA
            nc.sync.dma_start(
                arow[:].rearrange("h (t j k) -> h t j k", t=4, j=16),
                attn[:])

            # ---- weighted aggregation ----
            wacc = wacc_p.tile([128, D], f32)
            for t in range(TPB):
                ab = abps.tile([128, 512], f32)
                nc.tensor.matmul(ab[:], lhsT=(ones_t[:]),
                                 rhs=(arow[:, t * 512:(t + 1) * 512]),
                                 start=True, stop=True)
                wt = wsb_p.tile([128, 512], f32)
                nc.vector.tensor_tensor(out=wt[:], in0=euts[t][:], in1=ab[:],
                                        op=ALU.mult)
                nc.vector.tensor_reduce(
                    out=wacc[:, 16 * t:16 * (t + 1)],
                    in_=wt[:].rearrange("p (j k) -> p j k", j=16),
                    axis=AX.X, op=ALU.add)
            outp = qps.tile([128, 128], f32, tag="qp")
            nc.tensor.transpose(out=outp[0:64, :], in_=wacc[:], identity=id_t[:])
            osb = osb_p.tile([64, 128], f32)
            nc.scalar.copy(osb[:], outp[0:64, :])
            nc.sync.dma_start(
                outd[b * 128:(b + 1) * 128, :]
                    .rearrange("(pair n) d -> n pair d", pair=2),
                osb[:].rearrange("n (pair d) -> n pair d", pair=2))

    nc.compile()
    return nc


def _prep_host(nodes, neigh_idx, att1_w, att1_b, att2_w, att2_b, att3_w,
               nblk_per_core, u2e_f32):
    """Shard + reorder indices, build per-block compact tables + constants.
    Returns per-core maps (without the shared g2e table)."""
    npad = nblk_per_core * 128
    npc = min(NPC, npad)
    nodes = np.asarray(nodes).astype(np.int32)
    neigh = np.asarray(neigh_idx).astype(np.int32).reshape(-1, DEG)

    consts = {}
    att1_w = np.asarray(att1_w, np.float32)
    w1aT = att1_w[:, :D].T.copy()
    w1bT = att1_w[:, D:].T.copy()
    w2T = np.asarray(att2_w, np.float32).T.copy()

    def blockdiag(m):
        z = np.zeros((128, 128), np.float32)
        z[:64, :64] = m
        z[64:, 64:] = m
        return z

    import ml_dtypes
    bf = ml_dtypes.bfloat16
    consts["w1a"] = blockdiag(w1aT).astype(bf)
    consts["w1b"] = blockdiag(w1bT)
    consts["w2"] = blockdiag(w2T).astype(bf)
    # w3q[:, t*8 + (4h + t)] = w3 half-h; tile t's mm3 writes lg8 rows t, 4+t
    w3q = np.zeros((128, TPB, 8), np.float32)
    w3row = np.asarray(att3_w, np.float32)[0]
    for t in range(TPB):
        w3q[:64, t, t] = w3row
        w3q[64:, t, 4 + t] = w3row
    consts["w3q"] = w3q.reshape(128, TPB * 8).astype(bf)
    ones_bd = np.zeros((2, 128), np.float32)
    ones_bd[0, :64] = 1.0
    ones_bd[1, 64:] = 1.0
    consts["onesbd"] = ones_bd.astype(bf)
    # ind64[j, t*512 + e] = 1 iff j == 16t + e//32 (mm1b scatters per-node q)
    ind64 = np.zeros((64, TPB * 512), np.float32)
    for t in range(TPB):
        ind64[16 * t:16 * (t + 1), 512 * t:512 * (t + 1)] = np.repeat(
            np.eye(16, dtype=np.float32), 32, axis=1)
    consts["ind64"] = ind64.astype(bf)
    consts["b1st"] = np.tile(np.asarray(att1_b, np.float32), 2)[:, None].copy()
    consts["b2st"] = np.tile(np.asarray(att2_b, np.float32), 2)[:, None].copy()
    consts["ident"] = np.eye(128, dtype=np.float32)

    ncores = len(nodes) // npc if len(nodes) >= npc else 1
    per_core = []
    for c in range(ncores):
        n0 = c * npc
        nix = np.zeros((npad, DEG), np.int32)
        nix[:npc] = neigh[n0:n0 + npc]
        gid = np.zeros(npad, np.int32)
        gid[:npc] = nodes[n0:n0 + npc]
        # [b, n_local(128), k] -> [b, h, t, j, k] -> [b, t, h, j, k]
        a = nix.reshape(nblk_per_core, 2, TPB, 16, DEG).transpose(0, 2, 1, 3, 4)
        # gather position x = (t*8 + 2u + h)*128 + p; per-block global index
        # matrix eblk[b, p, t*8+2u+h]
        a = a.reshape(nblk_per_core, TPB, 2, 4, 128).transpose(0, 4, 1, 3, 2)
        eblk = a.reshape(nblk_per_core, 128, TPB * 8)
        # per-block compact table (unique u2e rows, f32) + local int16 indices
        ctab = np.zeros((nblk_per_core, EPB, D), np.float32)
        eidx16 = np.zeros((nblk_per_core, 128, EPB // 16), np.int16)
        for bb in range(nblk_per_core):
            uniq, inv = np.unique(eblk[bb], return_inverse=True)
            ctab[bb, :len(uniq)] = u2e_f32[uniq]
            inv = inv.reshape(128, TPB * 8).astype(np.int16)
            # position j = col*128 + p reads idx16[16r + j%16, j//16]
            loc = inv.T.reshape(-1)                       # loc[col*128+p]
            wrapped = loc.reshape(EPB // 16, 16).T        # [16, EPB//16]
            eidx16[bb] = np.tile(wrapped, (8, 1))
        # gidx row (b, p) = (node p, node 64+p) of block b
        gi2 = np.ascontiguousarray(
            gid.reshape(nblk_per_core, 2, 64).transpose(0, 2, 1)
               .reshape(nblk_per_core * 64, 2))
        m = dict(consts)
        m["ctab"] = ctab.reshape(nblk_per_core * EPB, D)
        m["eidx"] = eidx16.reshape(nblk_per_core * 128, EPB // 16)
        m["gidx"] = gi2
        per_core.append(m)
    return per_core


def kernel(nodes, neigh_idx, segment_ids, u2e_weight, g2e_weight,
           att1_w, att1_b, att2_w, att2_b, att3_w, att3_b):
    from concourse import bass_utils

    nblk = NPC // 128 + (1 if NPC % 128 else 0)  # 40
    key = ("prog", nblk)
    if key not in _cache:
        _cache[key] = _build_program(nblk)
    nc = _cache[key]

    u2e = np.ascontiguousarray(np.asarray(u2e_weight, np.float32))
    g2e = np.ascontiguousarray(np.asarray(g2e_weight, np.float32))
    per_core = _prep_host(nodes, neigh_idx, att1_w, att1_b, att2_w, att2_b,
                          att3_w, nblk, u2e)
    in_maps = []
    for m in per_core:
        m = dict(m)
        m["g2e"] = g2e
        in_maps.append(m)

    res = bass_utils.run_bass_kernel_spmd(nc, in_maps,
                                          core_ids=list(range(N_CORES)))
    outs = [np.asarray(r["out"])[:NPC] for r in res.results]
    return np.concatenate(outs, axis=0)



# revision 27
# speedup vs baseline: 1.1261x; 1.0487x over previous
# Trainium2 Bass kernel for nn_Member_Aggregator (GNN attention aggregation).
#
# Math (per edge e with node n = segment(e), 32 edges/node):
#   e_u   = u2e[neigh_idx]                          [E, 64]
#   g_rep = g2e[nodes][seg]                         [E, 64]
#   h1    = relu(e_u @ W1a.T + g_rep @ W1b.T + b1)  [E, 64]   (att1_w = [W1a | W1b])
#   h2    = relu(h1 @ W2.T + b2)                    [E, 64]
#   lg    = h2 @ w3.T (+ b3, dropped: softmax-invariant)
#   att   = segment_softmax(lg); out[n] = sum att * e_u        [N, 64]
#
# Sharding: 5000 contiguous nodes per core (x8), tables+weights replicated.
#
# Per-core layout ("stacked" feature-major): nodes padded to 5120 = 40 blocks
# x 128 nodes. Block = 4 tiles x 1024 edges. A tile pairs nodes {16t..16t+15}
# (top, SBUF partitions 0..63) with {64+16t..} (bottom, partitions 64..127),
# so every [128, 512] activation column holds one top edge + one bottom edge
# and all matmuls use block-diagonal weights at full 128-partition width.
# Edge slot x in [0,1024): x = c*128 + p (gather chunk c, partition p);
# top x = 32*j + k (node-slot j, neighbor k), bottom x-512 likewise.
#
# Edge embeddings are fetched with ONE dma_gather per block (4096 int16
# indices, vectorized Q7 descriptor generation) from a host-compacted
# per-block table (the block's <=4096 unique u2e rows, f32 so each row is a
# 256B gather element). This sidesteps both the 1-index-per-partition limit
# of indirect_dma_start and dma_gather's int16 index range.
#
# Per-edge q = g_rep @ W1b.T + b1 is folded into mm1 as extra contraction rows
# (lhsT = transposed per-node q, rhs = constant node-indicator), so no
# per-edge vector add is needed.

import os
import sys

import numpy as np

for _p in ("/opt/trn_rl_repo",):
    if _p not in sys.path:
        sys.path.insert(0, _p)

N_NODES = 40000
DEG = 32
D = 64
NUM_USERS = 100000
NUM_GROUPS = 50000
N_CORES = 8
NPC = N_NODES // N_CORES  # 5000 nodes per core
TPB = 4                   # tiles per block
EPT = 1024                # edges per tile
EPB = TPB * EPT           # 4096 edges per block (= compact table rows)

_cache = {}


def _build_program(nblk):
    """Build the SPMD per-core Bass program for `nblk` 128-node blocks."""
    import concourse.bass as bass
    import concourse.tile as tile
    from concourse import bacc, mybir
    from concourse.bass import IndirectOffsetOnAxis
    from contextlib import ExitStack

    f32 = mybir.dt.float32
    bf16 = mybir.dt.bfloat16
    i32 = mybir.dt.int32
    i16 = mybir.dt.int16
    AF = mybir.ActivationFunctionType
    ALU = mybir.AluOpType
    AX = mybir.AxisListType

    npad = nblk * 128
    ntile = nblk * TPB

    nc = bacc.Bacc("TRN2", target_bir_lowering=False, debug=False,
                   num_devices=N_CORES)

    ctab = nc.dram_tensor("ctab", [nblk * EPB, D], f32,
                          kind="ExternalInput").ap()
    g2e = nc.dram_tensor("g2e", [NUM_GROUPS, D], f32, kind="ExternalInput").ap()
    eidx = nc.dram_tensor("eidx", [nblk * 128, EPB // 16], i16,
                          kind="ExternalInput").ap()
    gidx = nc.dram_tensor("gidx", [nblk * 64, 2], i32, kind="ExternalInput").ap()
    w1a_d = nc.dram_tensor("w1a", [128, 128], bf16, kind="ExternalInput").ap()
    w1b_d = nc.dram_tensor("w1b", [128, 128], f32, kind="ExternalInput").ap()
    w2_d = nc.dram_tensor("w2", [128, 128], bf16, kind="ExternalInput").ap()
    w3_d = nc.dram_tensor("w3q", [128, TPB * 8], bf16, kind="ExternalInput").ap()
    ones_d = nc.dram_tensor("onesbd", [2, 128], bf16, kind="ExternalInput").ap()
    ind_d = nc.dram_tensor("ind64", [64, TPB * 512], bf16,
                           kind="ExternalInput").ap()
    b1_d = nc.dram_tensor("b1st", [128, 1], f32, kind="ExternalInput").ap()
    b2_d = nc.dram_tensor("b2st", [128, 1], f32, kind="ExternalInput").ap()
    id_d = nc.dram_tensor("ident", [128, 128], f32, kind="ExternalInput").ap()
    outd = nc.dram_tensor("out", [npad, D], f32, kind="ExternalOutput").ap()

    with tile.TileContext(nc) as tc, ExitStack() as ctx:
        cp = ctx.enter_context(tc.tile_pool(name="consts", bufs=1))

        def load_const(dram_ap, shape, tag, dt=f32):
            t = cp.tile(shape, dt, tag=tag)
            nc.sync.dma_start(t[:], dram_ap)
            return t

        w1a_t = load_const(w1a_d, [128, 128], "w1a", bf16)
        w1b_t = load_const(w1b_d, [128, 128], "w1b")
        w2_t = load_const(w2_d, [128, 128], "w2", bf16)
        w3_t = load_const(w3_d, [128, TPB * 8], "w3", bf16)
        ones_t = load_const(ones_d, [2, 128], "ones", bf16)
        ind_t = load_const(ind_d, [64, TPB * 512], "ind", bf16)
        b1_t = load_const(b1_d, [128, 1], "b1")
        b2_t = load_const(b2_d, [128, 1], "b2")
        id_t = load_const(id_d, [128, 128], "ident")

        gq = ctx.enter_context(tc.tile_pool(name="gq", bufs=2))
        qps = ctx.enter_context(tc.tile_pool(name="qpsum", bufs=2, space="PSUM"))
        eip = ctx.enter_context(tc.tile_pool(name="ei", bufs=3))
        gep = ctx.enter_context(tc.tile_pool(name="ge", bufs=3))
        tpps = ctx.enter_context(tc.tile_pool(name="tp", bufs=2, space="PSUM"))
        eut = ctx.enter_context(tc.tile_pool(name="eut", bufs=6))
        mmps = ctx.enter_context(tc.tile_pool(name="mm", bufs=2, space="PSUM"))
        hsb = ctx.enter_context(tc.tile_pool(name="h", bufs=3))
        lgps = ctx.enter_context(tc.tile_pool(name="lg", bufs=1, space="PSUM"))
        abps = ctx.enter_context(tc.tile_pool(name="attb", bufs=1, space="PSUM"))
        lrow_p = ctx.enter_context(tc.tile_pool(name="lrow", bufs=2))
        arow_p = ctx.enter_context(tc.tile_pool(name="arow", bufs=2))
        nm = ctx.enter_context(tc.tile_pool(name="nm", bufs=2))
        wsb_p = ctx.enter_context(tc.tile_pool(name="w", bufs=2))
        wacc_p = ctx.enter_context(tc.tile_pool(name="wacc", bufs=2))
        osb_p = ctx.enter_context(tc.tile_pool(name="osb", bufs=2))

        for b in range(nblk):
            # logits psum for the whole block: partition 2t+h = (tile t, half h)
            lg8 = lgps.tile([8, 512], f32)
            # ---- q phase: per-node q = g2e[gid] @ W1b.T + b1, transposed ----
            # gidx row p = (node p, node 64+p): gathered [64, 2, 64] chunk has
            # per-partition [g(n) | g(n+64)]; its transpose is the stacked g2T.
            gi = gq.tile([64, 2], i32, tag="gi")
            nc.sync.dma_start(gi[:], gidx[b * 64:(b + 1) * 64, :])
            gt = gq.tile([64, 128], f32, tag="gt")
            nc.gpsimd.indirect_dma_start(
                out=gt[:], out_offset=None, in_=g2e,
                in_offset=IndirectOffsetOnAxis(ap=gi[:], axis=0))
            g2T = qps.tile([128, 128], f32, tag="qp")
            nc.tensor.transpose(out=g2T[:, 0:64], in_=gt[:],
                                identity=id_t[0:64, 0:64])
            g2T_sb = gq.tile([128, D], f32, tag="g2Tsb")
            nc.scalar.copy(g2T_sb[:], g2T[:, 0:64])
            qp = qps.tile([128, 128], f32, tag="qp")
            nc.tensor.matmul(qp[:, 0:64], lhsT=w1b_t[:], rhs=g2T_sb[:],
                             start=True, stop=True)
            q2T_sb = gq.tile([128, D], f32, tag="q2T")
            nc.vector.tensor_scalar_add(q2T_sb[:], qp[:, 0:64], b1_t[:, :1])
            qT2p = qps.tile([128, 128], f32, tag="qp")
            nc.tensor.transpose(out=qT2p[0:64, :], in_=q2T_sb[:], identity=id_t[:])
            qT2_sb = gq.tile([64, 128], bf16, tag="qT2")
            nc.scalar.copy(qT2_sb[:], qT2p[0:64, :])

            # ---- edge phase: one dma_gather for the whole block ----
            ei = eip.tile([128, EPB // 16], i16)
            nc.sync.dma_start(ei[:], eidx[b * 128:(b + 1) * 128, :])
            ge = gep.tile([128, TPB * 512], f32)
            # single_packet caps at 64 descs/lane = 1024 idx -> 4 gathers;
            # multi-packet mode measured 8x slower (per-packet doorbells)
            for gsub in range(4):
                nc.gpsimd.dma_gather(
                    out_ap=ge[:, gsub * 512:(gsub + 1) * 512]
                        .rearrange("p (c f) -> p c f", f=D),
                    in_ap=ctab[b * EPB:(b + 1) * EPB, :],
                    idxs_ap=ei[:, gsub * 64:(gsub + 1) * 64],
                    num_idxs=EPT, num_idxs_reg=EPT, elem_size=D)
            euts = []
            for t in range(TPB):
                # chunks interleave (top, bottom) rows per partition, so each
                # [128, 128] transpose writes the stacked layout at base 0
                tp = tpps.tile([128, 512], f32)
                for u in range(4):
                    nc.tensor.transpose(
                        out=tp[:, 128 * u:128 * (u + 1)],
                        in_=ge[:, t * 512 + 128 * u:t * 512 + 128 * (u + 1)],
                        identity=id_t[:])
                eut_sb = eut.tile([128, 512], bf16)
                nc.scalar.copy(eut_sb[:], tp[:])
                euts.append(eut_sb)

                h1p = mmps.tile([128, 512], f32, tag="mm")
                nc.tensor.matmul(h1p[:], lhsT=(w1a_t[:]),
                                 rhs=(eut_sb[:]), start=True, stop=False)
                nc.tensor.matmul(h1p[:], lhsT=(qT2_sb[:]),
                                 rhs=(ind_t[:, t * 512:(t + 1) * 512]),
                                 start=False, stop=True)
                h1sb = hsb.tile([128, 512], bf16, tag="h")
                nc.scalar.activation(h1sb[:], h1p[:], AF.Relu)
                h2p = mmps.tile([128, 512], f32, tag="mm")
                nc.tensor.matmul(h2p[:], lhsT=(w2_t[:]),
                                 rhs=(h1sb[:]), start=True, stop=True)
                h2sb = hsb.tile([128, 512], bf16, tag="h")
                nc.scalar.activation(h2sb[:], h2p[:], AF.Relu, bias=b2_t[:, :1])
                nc.tensor.matmul(lg8[:], lhsT=(w3_t[:, 8 * t:8 * (t + 1)]),
                                 rhs=(h2sb[:]), start=(t == 0),
                                 stop=(t == TPB - 1))

            # ---- softmax over each node's 32 edges (node-major [128, 32]) ----
            lrow = lrow_p.tile([8, 512], f32)
            nc.scalar.copy(lrow[:], lg8[:])
            lnm = nm.tile([128, 32], f32, tag="lnm")
            # lg8 row q = 4h + t (set via w3q), so ravel orders line up:
            # lnm[64h+16t+j, k] = lrow[4h+t, 32j+k] in one partition-fan DMA
            nc.sync.dma_start(
                lnm[:],
                lrow[:].rearrange("q (j k) -> q j k", j=16))
            ngmax = nm.tile([128, 1], f32, tag="ngmax")
            nc.vector.tensor_reduce(out=ngmax[:], in_=lnm[:], axis=AX.X,
                                    op=ALU.max, negate=True)
            expn = nm.tile([128, 32], f32, tag="expn")
            sume = nm.tile([128, 1], f32, tag="sume")
            nc.scalar.activation(expn[:], lnm[:], AF.Exp, bias=ngmax[:, :1],
                                 accum_out=sume[:, :1])
            rinv = nm.tile([128, 1], f32, tag="rinv")
            nc.vector.reciprocal(rinv[:], sume[:])
            attn = nm.tile([128, 32], bf16, tag="attn")
            nc.vector.tensor_scalar_mul(attn[:], expn[:], rinv[:, :1])
            arow = arow_p.tile([2, TPB * 512], bf16)
            # arow[h, 512t+32j+k] = attn[64h+16t+j, k] in one partition-fan DMA
            nc.sync.dma_start(
                arow[:].rearrange("h (t j k) -> h t j k", t=4, j=16),
                attn[:])

            # ---- weighted aggregation ----
            wacc = wacc_p.tile([128, D], f32)
            for t in range(TPB):
                ab = abps.tile([128, 512], f32)
                nc.tensor.matmul(ab[:], lhsT=(ones_t[:]),
                                 rhs=(arow[:, t * 512:(t + 1) * 512]),
                                 start=True, stop=True)
                wt = wsb_p.tile([128, 512], f32)
                nc.vector.tensor_tensor(out=wt[:], in0=euts[t][:], in1=ab[:],
                                        op=ALU.mult)
                nc.vector.tensor_reduce(
                    out=wacc[:, 16 * t:16 * (t + 1)],
                    in_=wt[:].rearrange("p (j k) -> p j k", j=16),
                    axis=AX.X, op=ALU.add)
            outp = qps.tile([128, 128], f32, tag="qp")
            nc.tensor.transpose(out=outp[0:64, :], in_=wacc[:], identity=id_t[:])
            osb = osb_p.tile([64, 128], f32)
            nc.scalar.copy(osb[:], outp[0:64, :])
            nc.sync.dma_start(
                outd[b * 128:(b + 1) * 128, :]
                    .rearrange("(pair n) d -> n pair d", pair=2),
                osb[:].rearrange("n (pair d) -> n pair d", pair=2))

    nc.compile()
    return nc


def _prep_host(nodes, neigh_idx, att1_w, att1_b, att2_w, att2_b, att3_w,
               nblk_per_core, u2e_f32):
    """Shard + reorder indices, build per-block compact tables + constants.
    Returns per-core maps (without the shared g2e table)."""
    npad = nblk_per_core * 128
    npc = min(NPC, npad)
    nodes = np.asarray(nodes).astype(np.int32)
    neigh = np.asarray(neigh_idx).astype(np.int32).reshape(-1, DEG)

    consts = {}
    att1_w = np.asarray(att1_w, np.float32)
    w1aT = att1_w[:, :D].T.copy()
    w1bT = att1_w[:, D:].T.copy()
    w2T = np.asarray(att2_w, np.float32).T.copy()

    def blockdiag(m):
        z = np.zeros((128, 128), np.float32)
        z[:64, :64] = m
        z[64:, 64:] = m
        return z

    import ml_dtypes
    bf = ml_dtypes.bfloat16
    consts["w1a"] = blockdiag(w1aT).astype(bf)
    consts["w1b"] = blockdiag(w1bT)
    consts["w2"] = blockdiag(w2T).astype(bf)
    # w3q[:, t*8 + (4h + t)] = w3 half-h; tile t's mm3 writes lg8 rows t, 4+t
    w3q = np.zeros((128, TPB, 8), np.float32)
    w3row = np.asarray(att3_w, np.float32)[0]
    for t in range(TPB):
        w3q[:64, t, t] = w3row
        w3q[64:, t, 4 + t] = w3row
    consts["w3q"] = w3q.reshape(128, TPB * 8).astype(bf)
    ones_bd = np.zeros((2, 128), np.float32)
    ones_bd[0, :64] = 1.0
    ones_bd[1, 64:] = 1.0
    consts["onesbd"] = ones_bd.astype(bf)
    # ind64[j, t*512 + e] = 1 iff j == 16t + e//32 (mm1b scatters per-node q)
    ind64 = np.zeros((64, TPB * 512), np.float32)
    for t in range(TPB):
        ind64[16 * t:16 * (t + 1), 512 * t:512 * (t + 1)] = np.repeat(
            np.eye(16, dtype=np.float32), 32, axis=1)
    consts["ind64"] = ind64.astype(bf)
    consts["b1st"] = np.tile(np.asarray(att1_b, np.float32), 2)[:, None].copy()
    consts["b2st"] = np.tile(np.asarray(att2_b, np.float32), 2)[:, None].copy()
    consts["ident"] = np.eye(128, dtype=np.float32)

    ncores = len(nodes) // npc if len(nodes) >= npc else 1
    per_core = []
    for c in range(ncores):
        n0 = c * npc
        nix = np.zeros((npad, DEG), np.int32)
        nix[:npc] = neigh[n0:n0 + npc]
        gid = np.zeros(npad, np.int32)
        gid[:npc] = nodes[n0:n0 + npc]
        # [b, n_local(128), k] -> [b, h, t, j, k] -> [b, t, h, j, k]
        a = nix.reshape(nblk_per_core, 2, TPB, 16, DEG).transpose(0, 2, 1, 3, 4)
        # gather position x = (t*8 + 2u + h)*128 + p; per-block global index
        # matrix eblk[b, p, t*8+2u+h]
        a = a.reshape(nblk_per_core, TPB, 2, 4, 128).transpose(0, 4, 1, 3, 2)
        eblk = a.reshape(nblk_per_core, 128, TPB * 8)
        # per-block compact table (unique u2e rows, f32) + local int16 indices
        ctab = np.zeros((nblk_per_core, EPB, D), np.float32)
        eidx16 = np.zeros((nblk_per_core, 128, EPB // 16), np.int16)
        for bb in range(nblk_per_core):
            uniq, inv = np.unique(eblk[bb], return_inverse=True)
            ctab[bb, :len(uniq)] = u2e_f32[uniq]
            inv = inv.reshape(128, TPB * 8).astype(np.int16)
            # position j = col*128 + p reads idx16[16r + j%16, j//16]
            loc = inv.T.reshape(-1)                       # loc[col*128+p]
            wrapped = loc.reshape(EPB // 16, 16).T        # [16, EPB//16]
            eidx16[bb] = np.tile(wrapped, (8, 1))
        # gidx row (b, p) = (node p, node 64+p) of block b
        gi2 = np.ascontiguousarray(
            gid.reshape(nblk_per_core, 2, 64).transpose(0, 2, 1)
               .reshape(nblk_per_core * 64, 2))
        m = dict(consts)
        m["ctab"] = ctab.reshape(nblk_per_core * EPB, D)
        m["eidx"] = eidx16.reshape(nblk_per_core * 128, EPB // 16)
        m["gidx"] = gi2
        per_core.append(m)
    return per_core


def kernel(nodes, neigh_idx, segment_ids, u2e_weight, g2e_weight,
           att1_w, att1_b, att2_w, att2_b, att3_w, att3_b):
    from concourse import bass_utils

    nblk = NPC // 128 + (1 if NPC % 128 else 0)  # 40
    key = ("prog", nblk)
    if key not in _cache:
        _cache[key] = _build_program(nblk)
    nc = _cache[key]

    u2e = np.ascontiguousarray(np.asarray(u2e_weight, np.float32))
    g2e = np.ascontiguousarray(np.asarray(g2e_weight, np.float32))
    per_core = _prep_host(nodes, neigh_idx, att1_w, att1_b, att2_w, att2_b,
                          att3_w, nblk, u2e)
    in_maps = []
    for m in per_core:
        m = dict(m)
        m["g2e"] = g2e
        in_maps.append(m)

    res = bass_utils.run_bass_kernel_spmd(nc, in_maps,
                                          core_ids=list(range(N_CORES)))
    outs = [np.asarray(r["out"])[:NPC] for r in res.results]
    return np.concatenate(outs, axis=0)



# revision 29
# speedup vs baseline: 1.5857x; 1.4082x over previous
# Trainium2 Bass kernel for nn_Member_Aggregator (GNN attention aggregation).
#
# Math (per edge e with node n = segment(e), 32 edges/node):
#   e_u   = u2e[neigh_idx]                          [E, 64]
#   g_rep = g2e[nodes][seg]                         [E, 64]
#   h1    = relu(e_u @ W1a.T + g_rep @ W1b.T + b1)  [E, 64]   (att1_w = [W1a | W1b])
#   h2    = relu(h1 @ W2.T + b2)                    [E, 64]
#   lg    = h2 @ w3.T (+ b3, dropped: softmax-invariant)
#   att   = segment_softmax(lg); out[n] = sum att * e_u        [N, 64]
#
# Sharding: 5000 contiguous nodes per core (x8), tables+weights replicated.
#
# Per-core layout ("stacked" feature-major): nodes padded to 5120 = 40 blocks
# x 128 nodes. Block = 4 tiles x 1024 edges. A tile pairs nodes {16t..16t+15}
# (top, SBUF partitions 0..63) with {64+16t..} (bottom, partitions 64..127),
# so every [128, 512] activation column holds one top edge + one bottom edge
# and all matmuls use block-diagonal weights at full 128-partition width.
# Edge slot x in [0,1024): x = c*128 + p (gather chunk c, partition p);
# top x = 32*j + k (node-slot j, neighbor k), bottom x-512 likewise.
#
# Edge embeddings are fetched with ONE dma_gather per block (4096 int16
# indices, vectorized Q7 descriptor generation) from a host-compacted
# per-block table (the block's <=4096 unique u2e rows, f32 so each row is a
# 256B gather element). This sidesteps both the 1-index-per-partition limit
# of indirect_dma_start and dma_gather's int16 index range.
#
# Per-edge q = g_rep @ W1b.T + b1 is folded into mm1 as extra contraction rows
# (lhsT = transposed per-node q, rhs = constant node-indicator), so no
# per-edge vector add is needed.

import os
import sys

import numpy as np

for _p in ("/opt/trn_rl_repo",):
    if _p not in sys.path:
        sys.path.insert(0, _p)

N_NODES = 40000
DEG = 32
D = 64
NUM_USERS = 100000
NUM_GROUPS = 50000
N_CORES = 8
NPC = N_NODES // N_CORES  # 5000 nodes per core
TPB = 4                   # tiles per block
EPT = 1024                # edges per tile
EPB = TPB * EPT           # 4096 edges per block (= compact table rows)

_cache = {}


def _build_program(nblk):
    """Build the SPMD per-core Bass program for `nblk` 128-node blocks."""
    import concourse.bass as bass
    import concourse.tile as tile
    from concourse import bacc, mybir
    from concourse.bass import IndirectOffsetOnAxis
    from contextlib import ExitStack

    f32 = mybir.dt.float32
    bf16 = mybir.dt.bfloat16
    i32 = mybir.dt.int32
    i16 = mybir.dt.int16
    AF = mybir.ActivationFunctionType
    ALU = mybir.AluOpType
    AX = mybir.AxisListType

    npad = nblk * 128
    ntile = nblk * TPB

    nc = bacc.Bacc("TRN2", target_bir_lowering=False, debug=False,
                   num_devices=N_CORES, num_swdge_queues=4)

    ctab = nc.dram_tensor("ctab", [nblk * EPB, D], f32,
                          kind="ExternalInput").ap()
    g2e = nc.dram_tensor("g2e", [NUM_GROUPS, D], f32, kind="ExternalInput").ap()
    eidx = nc.dram_tensor("eidx", [nblk * 128, EPB // 16], i16,
                          kind="ExternalInput").ap()
    gidx = nc.dram_tensor("gidx", [nblk * 64, 2], i32, kind="ExternalInput").ap()
    w1a_d = nc.dram_tensor("w1a", [128, 128], bf16, kind="ExternalInput").ap()
    w1b_d = nc.dram_tensor("w1b", [128, 128], f32, kind="ExternalInput").ap()
    w2_d = nc.dram_tensor("w2", [128, 128], bf16, kind="ExternalInput").ap()
    w3_d = nc.dram_tensor("w3q", [128, TPB * 8], bf16, kind="ExternalInput").ap()
    ones_d = nc.dram_tensor("onesbd", [2, 128], bf16, kind="ExternalInput").ap()
    ind_d = nc.dram_tensor("ind64", [64, TPB * 512], bf16,
                           kind="ExternalInput").ap()
    b1_d = nc.dram_tensor("b1st", [128, 1], f32, kind="ExternalInput").ap()
    b2_d = nc.dram_tensor("b2st", [128, 1], f32, kind="ExternalInput").ap()
    id_d = nc.dram_tensor("ident", [128, 128], f32, kind="ExternalInput").ap()
    outd = nc.dram_tensor("out", [npad, D], f32, kind="ExternalOutput").ap()

    with tile.TileContext(nc) as tc, ExitStack() as ctx:
        cp = ctx.enter_context(tc.tile_pool(name="consts", bufs=1))

        def load_const(dram_ap, shape, tag, dt=f32):
            t = cp.tile(shape, dt, tag=tag)
            nc.sync.dma_start(t[:], dram_ap)
            return t

        w1a_t = load_const(w1a_d, [128, 128], "w1a", bf16)
        w1b_t = load_const(w1b_d, [128, 128], "w1b")
        w2_t = load_const(w2_d, [128, 128], "w2", bf16)
        w3_t = load_const(w3_d, [128, TPB * 8], "w3", bf16)
        ones_t = load_const(ones_d, [2, 128], "ones", bf16)
        ind_t = load_const(ind_d, [64, TPB * 512], "ind", bf16)
        b1_t = load_const(b1_d, [128, 1], "b1")
        b2_t = load_const(b2_d, [128, 1], "b2")
        id_t = load_const(id_d, [128, 128], "ident")

        gq = ctx.enter_context(tc.tile_pool(name="gq", bufs=2))
        qps = ctx.enter_context(tc.tile_pool(name="qpsum", bufs=2, space="PSUM"))
        eip = ctx.enter_context(tc.tile_pool(name="ei", bufs=3))
        gep = ctx.enter_context(tc.tile_pool(name="ge", bufs=3))
        tpps = ctx.enter_context(tc.tile_pool(name="tp", bufs=2, space="PSUM"))
        eut = ctx.enter_context(tc.tile_pool(name="eut", bufs=6))
        mmps = ctx.enter_context(tc.tile_pool(name="mm", bufs=2, space="PSUM"))
        hsb = ctx.enter_context(tc.tile_pool(name="h", bufs=3))
        lgps = ctx.enter_context(tc.tile_pool(name="lg", bufs=1, space="PSUM"))
        abps = ctx.enter_context(tc.tile_pool(name="attb", bufs=1, space="PSUM"))
        lrow_p = ctx.enter_context(tc.tile_pool(name="lrow", bufs=2))
        arow_p = ctx.enter_context(tc.tile_pool(name="arow", bufs=2))
        nm = ctx.enter_context(tc.tile_pool(name="nm", bufs=2))
        wsb_p = ctx.enter_context(tc.tile_pool(name="w", bufs=2))
        wacc_p = ctx.enter_context(tc.tile_pool(name="wacc", bufs=2))
        osb_p = ctx.enter_context(tc.tile_pool(name="osb", bufs=2))

        for b in range(nblk):
            # logits psum for the whole block: partition 2t+h = (tile t, half h)
            lg8 = lgps.tile([8, 512], f32)
            # ---- q phase: per-node q = g2e[gid] @ W1b.T + b1, transposed ----
            # gidx row p = (node p, node 64+p): gathered [64, 2, 64] chunk has
            # per-partition [g(n) | g(n+64)]; its transpose is the stacked g2T.
            gi = gq.tile([64, 2], i32, tag="gi")
            nc.sync.dma_start(gi[:], gidx[b * 64:(b + 1) * 64, :])
            gt = gq.tile([64, 128], f32, tag="gt")
            nc.gpsimd.indirect_dma_start(
                out=gt[:], out_offset=None, in_=g2e,
                in_offset=IndirectOffsetOnAxis(ap=gi[:], axis=0))
            g2T = qps.tile([128, 128], f32, tag="qp")
            nc.tensor.transpose(out=g2T[:, 0:64], in_=gt[:],
                                identity=id_t[0:64, 0:64])
            g2T_sb = gq.tile([128, D], f32, tag="g2Tsb")
            nc.scalar.copy(g2T_sb[:], g2T[:, 0:64])
            qp = qps.tile([128, 128], f32, tag="qp")
            nc.tensor.matmul(qp[:, 0:64], lhsT=w1b_t[:], rhs=g2T_sb[:],
                             start=True, stop=True)
            q2T_sb = gq.tile([128, D], f32, tag="q2T")
            nc.vector.tensor_scalar_add(q2T_sb[:], qp[:, 0:64], b1_t[:, :1])
            qT2p = qps.tile([128, 128], f32, tag="qp")
            nc.tensor.transpose(out=qT2p[0:64, :], in_=q2T_sb[:], identity=id_t[:])
            qT2_sb = gq.tile([64, 128], bf16, tag="qT2")
            nc.scalar.copy(qT2_sb[:], qT2p[0:64, :])

            # ---- edge phase: one dma_gather for the whole block ----
            ei = eip.tile([128, EPB // 16], i16)
            nc.sync.dma_start(ei[:], eidx[b * 128:(b + 1) * 128, :])
            ge = gep.tile([128, TPB * 512], f32)
            # single_packet caps at 64 descs/lane = 1024 idx -> 4 gathers;
            # multi-packet mode measured 8x slower (per-packet doorbells)
            # queue q is generated by Q7 core pair (2q, 2q+1): spreading the
            # four sub-gathers over queues 0-3 parallelizes Q7 desc-gen
            for gsub in range(4):
                nc.gpsimd.dma_gather(
                    out_ap=ge[:, gsub * 512:(gsub + 1) * 512]
                        .rearrange("p (c f) -> p c f", f=D),
                    in_ap=ctab[b * EPB:(b + 1) * EPB, :],
                    idxs_ap=ei[:, gsub * 64:(gsub + 1) * 64],
                    num_idxs=EPT, num_idxs_reg=EPT, elem_size=D,
                    queue_num=gsub)
            euts = []
            for t in range(TPB):
                # chunks interleave (top, bottom) rows per partition, so each
                # [128, 128] transpose writes the stacked layout at base 0
                tp = tpps.tile([128, 512], f32)
                for u in range(4):
                    nc.tensor.transpose(
                        out=tp[:, 128 * u:128 * (u + 1)],
                        in_=ge[:, t * 512 + 128 * u:t * 512 + 128 * (u + 1)],
                        identity=id_t[:])
                eut_sb = eut.tile([128, 512], bf16)
                nc.scalar.copy(eut_sb[:], tp[:])
                euts.append(eut_sb)

                h1p = mmps.tile([128, 512], f32, tag="mm")
                nc.tensor.matmul(h1p[:], lhsT=(w1a_t[:]),
                                 rhs=(eut_sb[:]), start=True, stop=False)
                nc.tensor.matmul(h1p[:], lhsT=(qT2_sb[:]),
                                 rhs=(ind_t[:, t * 512:(t + 1) * 512]),
                                 start=False, stop=True)
                h1sb = hsb.tile([128, 512], bf16, tag="h")
                nc.scalar.activation(h1sb[:], h1p[:], AF.Relu)
                h2p = mmps.tile([128, 512], f32, tag="mm")
                nc.tensor.matmul(h2p[:], lhsT=(w2_t[:]),
                                 rhs=(h1sb[:]), start=True, stop=True)
                h2sb = hsb.tile([128, 512], bf16, tag="h")
                nc.scalar.activation(h2sb[:], h2p[:], AF.Relu, bias=b2_t[:, :1])
                nc.tensor.matmul(lg8[:], lhsT=(w3_t[:, 8 * t:8 * (t + 1)]),
                                 rhs=(h2sb[:]), start=(t == 0),
                                 stop=(t == TPB - 1))

            # ---- softmax over each node's 32 edges (node-major [128, 32]) ----
            lrow = lrow_p.tile([8, 512], f32)
            nc.scalar.copy(lrow[:], lg8[:])
            lnm = nm.tile([128, 32], f32, tag="lnm")
            # lg8 row q = 4h + t (set via w3q), so ravel orders line up:
            # lnm[64h+16t+j, k] = lrow[4h+t, 32j+k] in one partition-fan DMA
            nc.sync.dma_start(
                lnm[:],
                lrow[:].rearrange("q (j k) -> q j k", j=16))
            ngmax = nm.tile([128, 1], f32, tag="ngmax")
            nc.vector.tensor_reduce(out=ngmax[:], in_=lnm[:], axis=AX.X,
                                    op=ALU.max, negate=True)
            expn = nm.tile([128, 32], f32, tag="expn")
            sume = nm.tile([128, 1], f32, tag="sume")
            nc.scalar.activation(expn[:], lnm[:], AF.Exp, bias=ngmax[:, :1],
                                 accum_out=sume[:, :1])
            rinv = nm.tile([128, 1], f32, tag="rinv")
            nc.vector.reciprocal(rinv[:], sume[:])
            attn = nm.tile([128, 32], bf16, tag="attn")
            nc.vector.tensor_scalar_mul(attn[:], expn[:], rinv[:, :1])
            arow = arow_p.tile([2, TPB * 512], bf16)
            # arow[h, 512t+32j+k] = attn[64h+16t+j, k] in one partition-fan DMA
            nc.sync.dma_start(
                arow[:].rearrange("h (t j k) -> h t j k", t=4, j=16),
                attn[:])

            # ---- weighted aggregation ----
            wacc = wacc_p.tile([128, D], f32)
            for t in range(TPB):
                ab = abps.tile([128, 512], f32)
                nc.tensor.matmul(ab[:], lhsT=(ones_t[:]),
                                 rhs=(arow[:, t * 512:(t + 1) * 512]),
                                 start=True, stop=True)
                wt = wsb_p.tile([128, 512], f32)
                nc.vector.tensor_tensor(out=wt[:], in0=euts[t][:], in1=ab[:],
                                        op=ALU.mult)
                nc.vector.tensor_reduce(
                    out=wacc[:, 16 * t:16 * (t + 1)],
                    in_=wt[:].rearrange("p (j k) -> p j k", j=16),
                    axis=AX.X, op=ALU.add)
            outp = qps.tile([128, 128], f32, tag="qp")
            nc.tensor.transpose(out=outp[0:64, :], in_=wacc[:], identity=id_t[:])
            osb = osb_p.tile([64, 128], f32)
            nc.scalar.copy(osb[:], outp[0:64, :])
            nc.sync.dma_start(
                outd[b * 128:(b + 1) * 128, :]
                    .rearrange("(pair n) d -> n pair d", pair=2),
                osb[:].rearrange("n (pair d) -> n pair d", pair=2))

    nc.compile()
    return nc


def _prep_host(nodes, neigh_idx, att1_w, att1_b, att2_w, att2_b, att3_w,
               nblk_per_core, u2e_f32):
    """Shard + reorder indices, build per-block compact tables + constants.
    Returns per-core maps (without the shared g2e table)."""
    npad = nblk_per_core * 128
    npc = min(NPC, npad)
    nodes = np.asarray(nodes).astype(np.int32)
    neigh = np.asarray(neigh_idx).astype(np.int32).reshape(-1, DEG)

    consts = {}
    att1_w = np.asarray(att1_w, np.float32)
    w1aT = att1_w[:, :D].T.copy()
    w1bT = att1_w[:, D:].T.copy()
    w2T = np.asarray(att2_w, np.float32).T.copy()

    def blockdiag(m):
        z = np.zeros((128, 128), np.float32)
        z[:64, :64] = m
        z[64:, 64:] = m
        return z

    import ml_dtypes
    bf = ml_dtypes.bfloat16
    consts["w1a"] = blockdiag(w1aT).astype(bf)
    consts["w1b"] = blockdiag(w1bT)
    consts["w2"] = blockdiag(w2T).astype(bf)
    # w3q[:, t*8 + (4h + t)] = w3 half-h; tile t's mm3 writes lg8 rows t, 4+t
    w3q = np.zeros((128, TPB, 8), np.float32)
    w3row = np.asarray(att3_w, np.float32)[0]
    for t in range(TPB):
        w3q[:64, t, t] = w3row
        w3q[64:, t, 4 + t] = w3row
    consts["w3q"] = w3q.reshape(128, TPB * 8).astype(bf)
    ones_bd = np.zeros((2, 128), np.float32)
    ones_bd[0, :64] = 1.0
    ones_bd[1, 64:] = 1.0
    consts["onesbd"] = ones_bd.astype(bf)
    # ind64[j, t*512 + e] = 1 iff j == 16t + e//32 (mm1b scatters per-node q)
    ind64 = np.zeros((64, TPB * 512), np.float32)
    for t in range(TPB):
        ind64[16 * t:16 * (t + 1), 512 * t:512 * (t + 1)] = np.repeat(
            np.eye(16, dtype=np.float32), 32, axis=1)
    consts["ind64"] = ind64.astype(bf)
    consts["b1st"] = np.tile(np.asarray(att1_b, np.float32), 2)[:, None].copy()
    consts["b2st"] = np.tile(np.asarray(att2_b, np.float32), 2)[:, None].copy()
    consts["ident"] = np.eye(128, dtype=np.float32)

    ncores = len(nodes) // npc if len(nodes) >= npc else 1
    per_core = []
    for c in range(ncores):
        n0 = c * npc
        nix = np.zeros((npad, DEG), np.int32)
        nix[:npc] = neigh[n0:n0 + npc]
        gid = np.zeros(npad, np.int32)
        gid[:npc] = nodes[n0:n0 + npc]
        # [b, n_local(128), k] -> [b, h, t, j, k] -> [b, t, h, j, k]
        a = nix.reshape(nblk_per_core, 2, TPB, 16, DEG).transpose(0, 2, 1, 3, 4)
        # gather position x = (t*8 + 2u + h)*128 + p; per-block global index
        # matrix eblk[b, p, t*8+2u+h]
        a = a.reshape(nblk_per_core, TPB, 2, 4, 128).transpose(0, 4, 1, 3, 2)
        eblk = a.reshape(nblk_per_core, 128, TPB * 8)
        # per-block compact table (unique u2e rows, f32) + local int16 indices
        ctab = np.zeros((nblk_per_core, EPB, D), np.float32)
        eidx16 = np.zeros((nblk_per_core, 128, EPB // 16), np.int16)
        for bb in range(nblk_per_core):
            uniq, inv = np.unique(eblk[bb], return_inverse=True)
            ctab[bb, :len(uniq)] = u2e_f32[uniq]
            inv = inv.reshape(128, TPB * 8).astype(np.int16)
            # position j = col*128 + p reads idx16[16r + j%16, j//16]
            loc = inv.T.reshape(-1)                       # loc[col*128+p]
            wrapped = loc.reshape(EPB // 16, 16).T        # [16, EPB//16]
            eidx16[bb] = np.tile(wrapped, (8, 1))
        # gidx row (b, p) = (node p, node 64+p) of block b
        gi2 = np.ascontiguousarray(
            gid.reshape(nblk_per_core, 2, 64).transpose(0, 2, 1)
               .reshape(nblk_per_core * 64, 2))
        m = dict(consts)
        m["ctab"] = ctab.reshape(nblk_per_core * EPB, D)
        m["eidx"] = eidx16.reshape(nblk_per_core * 128, EPB // 16)
        m["gidx"] = gi2
        per_core.append(m)
    return per_core


def kernel(nodes, neigh_idx, segment_ids, u2e_weight, g2e_weight,
           att1_w, att1_b, att2_w, att2_b, att3_w, att3_b):
    from concourse import bass_utils

    nblk = NPC // 128 + (1 if NPC % 128 else 0)  # 40
    key = ("prog", nblk)
    if key not in _cache:
        _cache[key] = _build_program(nblk)
    nc = _cache[key]

    u2e = np.ascontiguousarray(np.asarray(u2e_weight, np.float32))
    g2e = np.ascontiguousarray(np.asarray(g2e_weight, np.float32))
    per_core = _prep_host(nodes, neigh_idx, att1_w, att1_b, att2_w, att2_b,
                          att3_w, nblk, u2e)
    in_maps = []
    for m in per_core:
        m = dict(m)
        m["g2e"] = g2e
        in_maps.append(m)

    res = bass_utils.run_bass_kernel_spmd(nc, in_maps,
                                          core_ids=list(range(N_CORES)))
    outs = [np.asarray(r["out"])[:NPC] for r in res.results]
    return np.concatenate(outs, axis=0)



# revision 39
# speedup vs baseline: 1.9303x; 1.2173x over previous
# Trainium2 Bass kernel for nn_Member_Aggregator (GNN attention aggregation).
#
# Math (per edge e with node n = segment(e), 32 edges/node):
#   e_u   = u2e[neigh_idx]                          [E, 64]
#   g_rep = g2e[nodes][seg]                         [E, 64]
#   h1    = relu(e_u @ W1a.T + g_rep @ W1b.T + b1)  [E, 64]   (att1_w = [W1a | W1b])
#   h2    = relu(h1 @ W2.T + b2)                    [E, 64]
#   lg    = h2 @ w3.T (+ b3, dropped: softmax-invariant)
#   att   = segment_softmax(lg); out[n] = sum att * e_u        [N, 64]
#
# Sharding: 5000 contiguous nodes per core (x8), tables+weights replicated.
#
# Per-core layout ("stacked" feature-major): nodes padded to 5120 = 40 blocks
# x 128 nodes. Block = 4 tiles x 1024 edges. A tile pairs nodes {16t..16t+15}
# (top, SBUF partitions 0..63) with {64+16t..} (bottom, partitions 64..127),
# so every [128, 512] activation column holds one top edge + one bottom edge
# and all matmuls use block-diagonal weights at full 128-partition width.
# Edge slot x in [0,1024): x = c*128 + p (gather chunk c, partition p);
# top x = 32*j + k (node-slot j, neighbor k), bottom x-512 likewise.
#
# Edge embeddings are fetched with ONE dma_gather per block (4096 int16
# indices, vectorized Q7 descriptor generation) from a host-compacted
# per-block table (the block's <=4096 unique u2e rows, f32 so each row is a
# 256B gather element). This sidesteps both the 1-index-per-partition limit
# of indirect_dma_start and dma_gather's int16 index range.
#
# Per-edge q = g_rep @ W1b.T + b1 is folded into mm1 as extra contraction rows
# (lhsT = transposed per-node q, rhs = constant node-indicator), so no
# per-edge vector add is needed.

import os
import sys

import numpy as np

for _p in ("/opt/trn_rl_repo",):
    if _p not in sys.path:
        sys.path.insert(0, _p)

N_NODES = 40000
DEG = 32
D = 64
NUM_USERS = 100000
NUM_GROUPS = 50000
N_CORES = 8
NPC = N_NODES // N_CORES  # 5000 nodes per core
TPB = 4                   # tiles per block
EPT = 1024                # edges per tile
EPB = TPB * EPT           # 4096 edges per block (= compact table rows)

_cache = {}


def _build_program(nblk):
    """Build the SPMD per-core Bass program for `nblk` 128-node blocks."""
    import concourse.bass as bass
    import concourse.tile as tile
    from concourse import bacc, mybir
    from concourse.bass import IndirectOffsetOnAxis
    from contextlib import ExitStack

    f32 = mybir.dt.float32
    bf16 = mybir.dt.bfloat16
    i32 = mybir.dt.int32
    i16 = mybir.dt.int16
    AF = mybir.ActivationFunctionType
    ALU = mybir.AluOpType
    AX = mybir.AxisListType

    npad = nblk * 128
    ntile = nblk * TPB

    nc = bacc.Bacc("TRN2", target_bir_lowering=False, debug=False,
                   num_devices=N_CORES, num_swdge_queues=4)

    ctab = nc.dram_tensor("ctab", [nblk * EPB, D], f32,
                          kind="ExternalInput").ap()
    eidx = nc.dram_tensor("eidx", [nblk * 128, EPB // 16], i16,
                          kind="ExternalInput").ap()
    # host-staged per-node g rows, pre-transposed + pair-stacked:
    # gstk[b*128 + 64*half + f, j] = g2e[node (b,128*?,...)][f]  (see prep)
    gstk = nc.dram_tensor("gstk", [nblk * 128, 64], f32,
                          kind="ExternalInput").ap()
    w1a_d = nc.dram_tensor("w1a", [128, 128], bf16, kind="ExternalInput").ap()
    w1b_d = nc.dram_tensor("w1b", [128, 128], f32, kind="ExternalInput").ap()
    w2_d = nc.dram_tensor("w2", [128, 128], bf16, kind="ExternalInput").ap()
    w3_d = nc.dram_tensor("w3q", [128, TPB * 8], bf16, kind="ExternalInput").ap()
    ones_d = nc.dram_tensor("onesbd", [2, 128], bf16, kind="ExternalInput").ap()
    ind_d = nc.dram_tensor("ind64", [64, TPB * 512], bf16,
                           kind="ExternalInput").ap()
    b1_d = nc.dram_tensor("b1st", [128, 1], f32, kind="ExternalInput").ap()
    b2_d = nc.dram_tensor("b2st", [128, 1], f32, kind="ExternalInput").ap()
    id_d = nc.dram_tensor("ident", [128, 128], f32, kind="ExternalInput").ap()
    idb_d = nc.dram_tensor("identb", [128, 128], bf16, kind="ExternalInput").ap()
    outd = nc.dram_tensor("out", [npad, D], f32, kind="ExternalOutput").ap()

    with tile.TileContext(nc) as tc, ExitStack() as ctx:
        cp = ctx.enter_context(tc.tile_pool(name="consts", bufs=1))

        def load_const(dram_ap, shape, tag, dt=f32):
            t = cp.tile(shape, dt, tag=tag)
            nc.sync.dma_start(t[:], dram_ap)
            return t

        w1a_t = load_const(w1a_d, [128, 128], "w1a", bf16)
        w1b_t = load_const(w1b_d, [128, 128], "w1b")
        w2_t = load_const(w2_d, [128, 128], "w2", bf16)
        w3_t = load_const(w3_d, [128, TPB * 8], "w3", bf16)
        ones_t = load_const(ones_d, [2, 128], "ones", bf16)
        ind_t = load_const(ind_d, [64, TPB * 512], "ind", bf16)
        b1_t = load_const(b1_d, [128, 1], "b1")
        b2_t = load_const(b2_d, [128, 1], "b2")
        id_t = load_const(id_d, [128, 128], "ident")
        idb_t = load_const(idb_d, [128, 128], "identb", bf16)

        gq = ctx.enter_context(tc.tile_pool(name="gq", bufs=2))
        qps = ctx.enter_context(tc.tile_pool(name="qpsum", bufs=2, space="PSUM"))
        eip = ctx.enter_context(tc.tile_pool(name="ei", bufs=3))
        gep = ctx.enter_context(tc.tile_pool(name="ge", bufs=3))
        tpps = ctx.enter_context(tc.tile_pool(name="tp", bufs=2, space="PSUM"))
        eut = ctx.enter_context(tc.tile_pool(name="eut", bufs=6))
        mmps = ctx.enter_context(tc.tile_pool(name="mm", bufs=2, space="PSUM"))
        hsb = ctx.enter_context(tc.tile_pool(name="h", bufs=3))
        lgps = ctx.enter_context(tc.tile_pool(name="lg", bufs=1, space="PSUM"))
        abps = ctx.enter_context(tc.tile_pool(name="attb", bufs=1, space="PSUM"))
        lrow_p = ctx.enter_context(tc.tile_pool(name="lrow", bufs=2))
        arow_p = ctx.enter_context(tc.tile_pool(name="arow", bufs=2))
        nm = ctx.enter_context(tc.tile_pool(name="nm", bufs=2))
        wsb_p = ctx.enter_context(tc.tile_pool(name="w", bufs=2))
        wacc_p = ctx.enter_context(tc.tile_pool(name="wacc", bufs=2))
        osb_p = ctx.enter_context(tc.tile_pool(name="osb", bufs=2))

        for b in range(nblk):
            # logits psum for the whole block: partition 4h+t = (tile t, half h)
            lg8 = lgps.tile([8, 512], f32)
            # ---- q phase: per-node q = g2e[gid] @ W1b.T + b1, transposed ----
            # gstk block is the stacked feature-major g2T (host-staged)
            g2T_sb = gq.tile([128, D], f32, tag="g2Tsb")
            nc.sync.dma_start(g2T_sb[:], gstk[b * 128:(b + 1) * 128, :])
            qp = qps.tile([128, 128], f32, tag="qp")
            nc.tensor.matmul(qp[:, 0:64], lhsT=w1b_t[:], rhs=g2T_sb[:],
                             start=True, stop=True)
            q2T_sb = gq.tile([128, D], f32, tag="q2T")
            nc.vector.tensor_scalar_add(q2T_sb[:], qp[:, 0:64], b1_t[:, :1])
            qT2p = qps.tile([128, 128], f32, tag="qp")
            nc.tensor.transpose(out=qT2p[0:64, :], in_=q2T_sb[:], identity=id_t[:])
            qT2_sb = gq.tile([64, 128], bf16, tag="qT2")
            nc.scalar.copy(qT2_sb[:], qT2p[0:64, :])

            # ---- edge phase: one dma_gather for the whole block ----
            ei = eip.tile([128, EPB // 16], i16)
            nc.sync.dma_start(ei[:], eidx[b * 128:(b + 1) * 128, :])
            ge = gep.tile([128, TPB * 512], f32, tag="gef")
            # single_packet caps at 64 descs/lane = 1024 idx -> 4 gathers;
            # multi-packet mode measured 8x slower (per-packet doorbells)
            # queue q is generated by Q7 core pair (2q, 2q+1): spreading the
            # four sub-gathers over queues 0-3 parallelizes Q7 desc-gen
            for gsub in range(4):
                nc.gpsimd.dma_gather(
                    out_ap=ge[:, gsub * 512:(gsub + 1) * 512]
                        .rearrange("p (c f) -> p c f", f=D),
                    in_ap=ctab[b * EPB:(b + 1) * EPB, :],
                    idxs_ap=ei[:, gsub * 64:(gsub + 1) * 64],
                    num_idxs=EPT, num_idxs_reg=EPT, elem_size=D,
                    queue_num=gsub)
            geb = gep.tile([128, TPB * 512], bf16, tag="geb")
            nc.vector.tensor_copy(geb[:], ge[:])
            euts = []
            for t in range(TPB):
                # chunks interleave (top, bottom) rows per partition, so each
                # [128, 128] transpose writes the stacked layout at base 0
                tp = tpps.tile([128, 512], bf16)
                for u in range(4):
                    nc.tensor.transpose(
                        out=tp[:, 128 * u:128 * (u + 1)],
                        in_=geb[:, t * 512 + 128 * u:t * 512 + 128 * (u + 1)],
                        identity=idb_t[:])
                eut_sb = eut.tile([128, 512], bf16)
                nc.scalar.copy(eut_sb[:], tp[:])
                euts.append(eut_sb)

                h1p = mmps.tile([128, 512], f32, tag="mm")
                nc.tensor.matmul(h1p[:], lhsT=(w1a_t[:]),
                                 rhs=(eut_sb[:]), start=True, stop=False)
                nc.tensor.matmul(h1p[:], lhsT=(qT2_sb[:]),
                                 rhs=(ind_t[:, t * 512:(t + 1) * 512]),
                                 start=False, stop=True)
                h1sb = hsb.tile([128, 512], bf16, tag="h")
                nc.scalar.activation(h1sb[:], h1p[:], AF.Relu)
                h2p = mmps.tile([128, 512], f32, tag="mm")
                nc.tensor.matmul(h2p[:], lhsT=(w2_t[:]),
                                 rhs=(h1sb[:]), start=True, stop=True)
                h2sb = hsb.tile([128, 512], bf16, tag="h")
                nc.scalar.activation(h2sb[:], h2p[:], AF.Relu, bias=b2_t[:, :1])
                nc.tensor.matmul(lg8[:], lhsT=(w3_t[:, 8 * t:8 * (t + 1)]),
                                 rhs=(h2sb[:]), start=(t == 0),
                                 stop=(t == TPB - 1))

            # ---- softmax over each node's 32 edges (node-major [128, 32]) ----
            lrow = lrow_p.tile([8, 512], f32)
            nc.scalar.copy(lrow[:], lg8[:])
            lnm = nm.tile([128, 32], f32, tag="lnm")
            # lg8 row q = 4h + t (set via w3q), so ravel orders line up:
            # lnm[64h+16t+j, k] = lrow[4h+t, 32j+k] in one partition-fan DMA
            nc.sync.dma_start(
                lnm[:],
                lrow[:].rearrange("q (j k) -> q j k", j=16))
            ngmax = nm.tile([128, 1], f32, tag="ngmax")
            nc.vector.tensor_reduce(out=ngmax[:], in_=lnm[:], axis=AX.X,
                                    op=ALU.max, negate=True)
            expn = nm.tile([128, 32], f32, tag="expn")
            sume = nm.tile([128, 1], f32, tag="sume")
            nc.scalar.activation(expn[:], lnm[:], AF.Exp, bias=ngmax[:, :1],
                                 accum_out=sume[:, :1])
            rinv = nm.tile([128, 1], f32, tag="rinv")
            nc.vector.reciprocal(rinv[:], sume[:])
            attn = nm.tile([128, 32], bf16, tag="attn")
            nc.vector.tensor_scalar_mul(attn[:], expn[:], rinv[:, :1])
            arow = arow_p.tile([2, TPB * 512], bf16)
            # arow[h, 512t+32j+k] = attn[64h+16t+j, k] in one partition-fan DMA
            nc.sync.dma_start(
                arow[:].rearrange("h (t j k) -> h t j k", t=4, j=16),
                attn[:])

            # ---- weighted aggregation ----
            wacc = wacc_p.tile([128, D], f32)
            for t in range(TPB):
                ab = abps.tile([128, 512], f32)
                nc.tensor.matmul(ab[:], lhsT=(ones_t[:]),
                                 rhs=(arow[:, t * 512:(t + 1) * 512]),
                                 start=True, stop=True)
                wt = wsb_p.tile([128, 512], f32)
                nc.vector.tensor_tensor(out=wt[:], in0=euts[t][:], in1=ab[:],
                                        op=ALU.mult)
                nc.vector.tensor_reduce(
                    out=wacc[:, 16 * t:16 * (t + 1)],
                    in_=wt[:].rearrange("p (j k) -> p j k", j=16),
                    axis=AX.X, op=ALU.add)
            outp = qps.tile([128, 128], f32, tag="qp")
            nc.tensor.transpose(out=outp[0:64, :], in_=wacc[:], identity=id_t[:])
            osb = osb_p.tile([64, 128], f32)
            nc.scalar.copy(osb[:], outp[0:64, :])
            nc.sync.dma_start(
                outd[b * 128:(b + 1) * 128, :]
                    .rearrange("(pair n) d -> n pair d", pair=2),
                osb[:].rearrange("n (pair d) -> n pair d", pair=2))

    nc.compile()
    return nc


def _prep_host(nodes, neigh_idx, att1_w, att1_b, att2_w, att2_b, att3_w,
               nblk_per_core, u2e_f32, g2e_f32):
    """Shard + reorder indices, build per-block compact tables + constants.
    Returns complete per-core input maps."""
    npad = nblk_per_core * 128
    npc = min(NPC, npad)
    nodes = np.asarray(nodes).astype(np.int32)
    neigh = np.asarray(neigh_idx).astype(np.int32).reshape(-1, DEG)

    consts = {}
    att1_w = np.asarray(att1_w, np.float32)
    w1aT = att1_w[:, :D].T.copy()
    w1bT = att1_w[:, D:].T.copy()
    w2T = np.asarray(att2_w, np.float32).T.copy()

    def blockdiag(m):
        z = np.zeros((128, 128), np.float32)
        z[:64, :64] = m
        z[64:, 64:] = m
        return z

    import ml_dtypes
    bf = ml_dtypes.bfloat16
    consts["w1a"] = blockdiag(w1aT).astype(bf)
    consts["w1b"] = blockdiag(w1bT)
    consts["w2"] = blockdiag(w2T).astype(bf)
    # w3q[:, t*8 + (4h + t)] = w3 half-h; tile t's mm3 writes lg8 rows t, 4+t
    w3q = np.zeros((128, TPB, 8), np.float32)
    w3row = np.asarray(att3_w, np.float32)[0]
    for t in range(TPB):
        w3q[:64, t, t] = w3row
        w3q[64:, t, 4 + t] = w3row
    consts["w3q"] = w3q.reshape(128, TPB * 8).astype(bf)
    ones_bd = np.zeros((2, 128), np.float32)
    ones_bd[0, :64] = 1.0
    ones_bd[1, 64:] = 1.0
    consts["onesbd"] = ones_bd.astype(bf)
    # ind64[j, t*512 + e] = 1 iff j == 16t + e//32 (mm1b scatters per-node q)
    ind64 = np.zeros((64, TPB * 512), np.float32)
    for t in range(TPB):
        ind64[16 * t:16 * (t + 1), 512 * t:512 * (t + 1)] = np.repeat(
            np.eye(16, dtype=np.float32), 32, axis=1)
    consts["ind64"] = ind64.astype(bf)
    consts["b1st"] = np.tile(np.asarray(att1_b, np.float32), 2)[:, None].copy()
    consts["b2st"] = np.tile(np.asarray(att2_b, np.float32), 2)[:, None].copy()
    consts["ident"] = np.eye(128, dtype=np.float32)
    consts["identb"] = np.eye(128, dtype=np.float32).astype(bf)

    ncores = len(nodes) // npc if len(nodes) >= npc else 1
    per_core = []
    for c in range(ncores):
        n0 = c * npc
        nix = np.zeros((npad, DEG), np.int32)
        nix[:npc] = neigh[n0:n0 + npc]
        gid = np.zeros(npad, np.int32)
        gid[:npc] = nodes[n0:n0 + npc]
        # [b, n_local(128), k] -> [b, h, t, j, k] -> [b, t, h, j, k]
        a = nix.reshape(nblk_per_core, 2, TPB, 16, DEG).transpose(0, 2, 1, 3, 4)
        # gather position x = (t*8 + 2u + h)*128 + p; per-block global index
        # matrix eblk[b, p, t*8+2u+h]
        a = a.reshape(nblk_per_core, TPB, 2, 4, 128).transpose(0, 4, 1, 3, 2)
        eblk = a.reshape(nblk_per_core, 128, TPB * 8)
        # per-block compact table (unique u2e rows, f32) + local int16 indices
        ctab = np.zeros((nblk_per_core, EPB, D), np.float32)
        eidx16 = np.zeros((nblk_per_core, 128, EPB // 16), np.int16)
        for bb in range(nblk_per_core):
            uniq, inv = np.unique(eblk[bb], return_inverse=True)
            ctab[bb, :len(uniq)] = u2e_f32[uniq]
            inv = inv.reshape(128, TPB * 8).astype(np.int16)
            # position j = col*128 + p reads idx16[16r + j%16, j//16]
            loc = inv.T.reshape(-1)                       # loc[col*128+p]
            wrapped = loc.reshape(EPB // 16, 16).T        # [16, EPB//16]
            eidx16[bb] = np.tile(wrapped, (8, 1))
        # gstk: stacked feature-major g2T per block (g2T[64c+f, p] =
        # g2e[node (b, 64c+p), f]) staged on host
        G = g2e_f32[gid].reshape(nblk_per_core, 2, 64, D)
        gstk = np.ascontiguousarray(
            G.transpose(0, 1, 3, 2).reshape(nblk_per_core * 128, D))
        m = dict(consts)
        m["ctab"] = ctab.reshape(nblk_per_core * EPB, D)
        m["eidx"] = eidx16.reshape(nblk_per_core * 128, EPB // 16)
        m["gstk"] = gstk
        per_core.append(m)
    return per_core


def kernel(nodes, neigh_idx, segment_ids, u2e_weight, g2e_weight,
           att1_w, att1_b, att2_w, att2_b, att3_w, att3_b):
    from concourse import bass_utils

    nblk = NPC // 128 + (1 if NPC % 128 else 0)  # 40
    key = ("prog", nblk)
    if key not in _cache:
        _cache[key] = _build_program(nblk)
    nc = _cache[key]

    u2e = np.ascontiguousarray(np.asarray(u2e_weight, np.float32))
    g2e = np.ascontiguousarray(np.asarray(g2e_weight, np.float32))
    in_maps = _prep_host(nodes, neigh_idx, att1_w, att1_b, att2_w, att2_b,
                         att3_w, nblk, u2e, g2e)

    res = bass_utils.run_bass_kernel_spmd(nc, in_maps,
                                          core_ids=list(range(N_CORES)))
    outs = [np.asarray(r["out"])[:NPC] for r in res.results]
    return np.concatenate(outs, axis=0)

